# revision 15
# baseline (speedup 1.0000x reference)
"""GraphSAGE (2-layer, mean aggregation) on 8 Trainium2 NeuronCores.

Strategy: destination nodes are sharded across the 8 cores (49 tiles of 128
nodes per core, LPT-balanced by in-degree). Edges are partitioned by
destination tile; per-edge source rows are fetched with batched SWDGE
dma_gather instructions (thousands of rows per instruction, int16 indices,
table split at row 32768 to satisfy the int16 range). Messages and selection
matrices are bf16. The segment sum for a destination tile is computed
feature-major on the PE as sum_c msgs_c^T @ S_c (messages stationary,
0/1 selection streaming), so no transposes are needed anywhere; the 1/deg
mean scaling is a single small elementwise multiply against a host-replicated
reciprocal tile. Layer 1 also pre-computes z = W2l@h and r = W2r@h + b2 so
that layer 2 only gathers 256-byte z rows (4x less traffic than h rows),
segment-sums them, scales and adds r. All float math runs on-device; the
host does integer index preprocessing, sharding/layout, and un-sharding.
"""
import heapq
import sys
from contextlib import ExitStack

import numpy as np
import ml_dtypes

for _p in ("/opt/trn_rl_repo",):
    if _p not in sys.path:
        sys.path.insert(0, _p)

import concourse.bass as bass
import concourse.tile as tile
from concourse import bacc, mybir
from concourse.bass_utils import run_bass_kernel_spmd


def _ensure_axon_hooks():
    """run_bass_kernel_spmd(trace=True) imports antenv.axon_hooks, which this
    image lacks; install a ctypes-backed hook so tracing works (or degrades
    to a no-op instead of an ImportError)."""
    try:
        import antenv.axon_hooks  # noqa: F401
        return
    except ImportError:
        pass
    import contextlib
    import ctypes
    import types

    def _make_hook():
        try:
            lib = ctypes.CDLL("/opt/axon/libaxon_pjrt.so")
        except OSError:
            return None
        if not hasattr(lib, "axon_start_nrt_profile"):
            return None
        lib.axon_start_nrt_profile.argtypes = [ctypes.POINTER(ctypes.c_int64), ctypes.c_size_t]
        lib.axon_start_nrt_profile.restype = ctypes.c_int64
        lib.axon_stop_nrt_profile.argtypes = [ctypes.c_char_p]
        lib.axon_stop_nrt_profile.restype = ctypes.c_int64

        @contextlib.contextmanager
        def _hook(output_dir, device_ids):
            import jax
            jax.devices()
            if device_ids:
                ids = (ctypes.c_int64 * len(device_ids))(*device_ids)
                rc = lib.axon_start_nrt_profile(ids, len(device_ids))
            else:
                rc = lib.axon_start_nrt_profile(None, 0)
            if rc != 0:
                raise RuntimeError(f"axon_start_nrt_profile rc={rc}")
            try:
                yield
            finally:
                lib.axon_stop_nrt_profile(str(output_dir).encode())

        return _hook

    hook = _make_hook()
    mod = types.ModuleType("antenv.axon_hooks")
    mod.get_axon_ntff_profile_hook = lambda: hook
    mod.set_axon_ntff_profile_hook = lambda h: None
    import antenv
    antenv.axon_hooks = mod
    sys.modules["antenv.axon_hooks"] = mod


_ensure_axon_hooks()


def _run_spmd_retry(nc, in_maps, **kw):
    """One retry for transient NRT device errors."""
    import time
    try:
        return run_bass_kernel_spmd(nc, in_maps, core_ids=list(range(N_CORES)), **kw)
    except Exception:
        time.sleep(15)
        return run_bass_kernel_spmd(nc, in_maps, core_ids=list(range(N_CORES)), **kw)


N_NODES = 50000
N_EDGES = 800000
DIM_IN, DIM_H, DIM_OUT = 128, 256, 64
N_CORES = 8
P = 128
TILES_PER_CORE = 49                      # ceil(50000 / 8 / 128)
N_TILES = N_CORES * TILES_PER_CORE       # 392
NPAD_CORE = TILES_PER_CORE * P           # 6272
NPAD = N_CORES * NPAD_CORE               # 50176
SPLIT = 32768                            # int16 gather index boundary
SENT = 200.0                             # dst_rel sentinel: matches no slot
GROUPS = (8, 8, 8, 8, 8, 8, 1)           # dst tiles per dma_gather group
GOFF = tuple(sum(GROUPS[:i]) for i in range(len(GROUPS)))

BF16 = ml_dtypes.bfloat16
LAST_RESULTS = []   # test harness reads profiling results from here


def _partition_nodes(deg):
    """LPT-pack nodes into N_TILES bins of <=128 nodes, minimizing max bin
    in-degree sum. Returns (tile_of, slot_of)."""
    order = np.argsort(-deg, kind="stable")
    heap = [(0, t) for t in range(N_TILES)]
    heapq.heapify(heap)
    counts = np.zeros(N_TILES, np.int64)
    sums = np.zeros(N_TILES, np.int64)
    tile_of = np.empty(N_NODES, np.int64)
    slot_of = np.empty(N_NODES, np.int64)
    for node in order:
        while True:
            s, t = heapq.heappop(heap)
            if counts[t] < P:
                break
        tile_of[node] = t
        slot_of[node] = counts[t]
        counts[t] += 1
        sums[t] += deg[node]
        if counts[t] < P:
            heapq.heappush(heap, (sums[t], t))
    return tile_of, slot_of


def _edge_layout(src_pos, dst, tile_of, slot_of):
    """Chunked per-tile edge layout with lo/hi (src_pos < SPLIT) segregation.

    Returns C_LO, C_HI and per-core (idx_lo, idx_hi, dst_rel):
      idx_lo [128, 49*C_LO*8] int16 -- per 7-tile group, the flat padded lo
        index list wrapped into 16 rows and replicated to 128 partitions
      idx_hi analogous with indices rebased by SPLIT
      dst_rel [128, 49*NCH] -- dst slot (or SENT) for edge at
        (partition, tile*NCH + chunk); lo chunks first, then hi
    """
    etile = tile_of[dst]
    ishi = (src_pos >= SPLIT).astype(np.int64)
    key = etile * 2 + ishi
    order = np.argsort(key, kind="stable")
    cnt = np.bincount(key, minlength=N_TILES * 2)
    lo_cnt, hi_cnt = cnt[0::2], cnt[1::2]
    C_LO = int(np.ceil(lo_cnt.max() / P))
    C_HI = int(np.ceil(hi_cnt.max() / P))
    NCH = C_LO + C_HI
    starts = np.concatenate([[0], np.cumsum(cnt)[:-1]])
    rank = np.arange(N_EDGES) - np.repeat(starts, cnt)
    es, ed, ek = src_pos[order], dst[order], key[order]
    ehalf, etl = ek & 1, ek >> 1
    ppos = np.where(ehalf == 0, rank, C_LO * P + rank)
    idx_pad = np.zeros((N_TILES, NCH * P), np.int64)
    dr_pad = np.full((N_TILES, NCH * P), SENT, np.float32)
    idx_pad[etl, ppos] = np.where(ehalf == 0, es, es - SPLIT)
    dr_pad[etl, ppos] = slot_of[ed]

    idx_los, idx_his, dst_rels = [], [], []

    def _wrap(rows):
        # Q7 desc-gen reads index position m from sbuf[m % 16, m // 16]
        blocks = []
        for g, gsz in enumerate(GROUPS):
            flat = rows[GOFF[g]:GOFF[g] + gsz].reshape(-1)
            blocks.append(flat.reshape(-1, 16).T)
        return np.tile(np.hstack(blocks), (8, 1)).astype(np.int16)

    for c in range(N_CORES):
        sl = slice(c * TILES_PER_CORE, (c + 1) * TILES_PER_CORE)
        ip, dp = idx_pad[sl], dr_pad[sl]
        idx_los.append(np.ascontiguousarray(_wrap(ip[:, :C_LO * P])))
        idx_his.append(np.ascontiguousarray(_wrap(ip[:, C_LO * P:])))
        dr3 = dp.reshape(TILES_PER_CORE, NCH, P)
        dst_rels.append(np.ascontiguousarray(
            dr3.transpose(2, 0, 1).reshape(P, TILES_PER_CORE * NCH).astype(BF16)))
    return C_LO, C_HI, idx_los, idx_his, dst_rels


def _seg_inputs(nc, dt, C_LO, C_HI):
    """Declare the gather/segment-sum inputs shared by both layer programs."""
    NCH = C_LO + C_HI
    return dict(
        idx_lo=nc.dram_tensor("idx_lo", [P, TILES_PER_CORE * C_LO * 8], dt.int16,
                              kind="ExternalInput").ap(),
        idx_hi=nc.dram_tensor("idx_hi", [P, TILES_PER_CORE * C_HI * 8], dt.int16,
                              kind="ExternalInput").ap(),
        dst_rel=nc.dram_tensor("dst_rel", [P, TILES_PER_CORE * NCH], dt.bfloat16,
                               kind="ExternalInput").ap(),
        recip=nc.dram_tensor("recip", [P, NPAD_CORE], dt.float32,
                             kind="ExternalInput").ap(),
        iota=nc.dram_tensor("iota", [P, NCH * P], dt.bfloat16,
                            kind="ExternalInput").ap(),
    )


def _load_seg_consts(nc, const, dt, ins, C_LO, C_HI):
    NCH = C_LO + C_HI
    sb = {}
    # idx loads split per group so the first gather starts early
    for name, cpt in (("idx_lo", C_LO * 8), ("idx_hi", C_HI * 8)):
        t = const.tile([P, TILES_PER_CORE * cpt], dt.int16, name=name + "_sb")
        for g, gsz in enumerate(GROUPS):
            c0, c1 = GOFF[g] * cpt, (GOFF[g] + gsz) * cpt
            nc.sync.dma_start(t[:, c0:c1], ins[name][:, c0:c1])
        sb[name] = t
    for name, shape, d in (
        ("dst_rel", [P, TILES_PER_CORE * NCH], dt.bfloat16),
        ("recip", [P, NPAD_CORE], dt.float32),
        ("iota", [P, NCH * P], dt.bfloat16),
    ):
        t = const.tile(shape, d, name=name + "_sb")
        nc.sync.dma_start(t[:], ins[name][:, :])
        sb[name] = t
    return sb


CPI = 8   # chunks per dma_gather: HW caps num_idxs at 1024 (16ch x 64)


_QCTR = [0]


def _gather_span(nc, msgs, idx_sb, table, col0, n_chunks):
    """Gather n_chunks*128 rows into msgs (split into <=1024-idx instrs,
    rotated over the 4 SWDGE queues so all four Q7 pairs generate
    descriptors concurrently). Column-slicing the idx block preserves the
    16-way interleaved wrap."""
    view = msgs[:].rearrange("p (c n) -> p c n", n=P)
    for s in range(0, n_chunks, CPI):
        e = min(s + CPI, n_chunks)
        n = (e - s) * P
        nc.gpsimd.dma_gather(
            view[:, s:e, :], table[:, :],
            idx_sb[:, col0 + s * 8:col0 + e * 8], n, n, P,
            queue_num=_QCTR[0] % 4)
        _QCTR[0] += 1


def _gather_group(nc, msgs_lo, msgs_hi, table_lo, table_hi, sb, g, C_LO, C_HI):
    """Issue the batched gathers for tile group g."""
    gsz = GROUPS[g]
    _gather_span(nc, msgs_lo, sb["idx_lo"], table_lo, GOFF[g] * C_LO * 8, gsz * C_LO)
    _gather_span(nc, msgs_hi, sb["idx_hi"], table_hi, GOFF[g] * C_HI * 8, gsz * C_HI)


def _build_S(nc, S, sb, tt, C_LO, C_HI):
    """S[e, c*128+n] = 1.0 if edge (partition e, chunk c) of tile tt targets
    slot n else 0.0; built in one DVE op (fallback: per-chunk)."""
    NCH = C_LO + C_HI
    dr = sb["dst_rel"]
    iota = sb["iota"]
    try:
        nc.vector.tensor_tensor(
            out=S[:],
            in0=dr[:, tt * NCH:(tt + 1) * NCH, None].to_broadcast([P, NCH, P]),
            in1=iota[:],
            op=mybir.AluOpType.is_equal,
        )
    except Exception:
        for cch in range(NCH):
            nc.vector.tensor_tensor(
                out=S[:, cch * P:(cch + 1) * P],
                in0=dr[:, tt * NCH + cch:tt * NCH + cch + 1].to_broadcast([P, P]),
                in1=iota[:, :P],
                op=mybir.AluOpType.is_equal,
            )


def _seg_matmuls(nc, out_ps, msgs_lo, msgs_hi, S, j, C_LO, C_HI, f_width):
    """out_ps[f, n] = sum_e msgs[e, f] * S[e, n] accumulated over all chunks
    of tile j-within-group. msgs chunks are the PE stationary operand."""
    NCH = C_LO + C_HI
    k = 0
    for c in range(C_LO):
        nc.tensor.matmul(
            out=out_ps[:],
            lhsT=msgs_lo[:, (j * C_LO + c) * P:(j * C_LO + c) * P + f_width],
            rhs=S[:, c * P:(c + 1) * P],
            start=(k == 0), stop=(k == NCH - 1))
        k += 1
    for c in range(C_HI):
        nc.tensor.matmul(
            out=out_ps[:],
            lhsT=msgs_hi[:, (j * C_HI + c) * P:(j * C_HI + c) * P + f_width],
            rhs=S[:, (C_LO + c) * P:(C_LO + c + 1) * P],
            start=(k == 0), stop=(k == NCH - 1))
        k += 1


def _build_prog1(C_LO, C_HI):
    """Layer 1 + pre-projection for layer 2.

    Per tile: gather x rows (bf16), segment-sum feature-major, mean-scale,
    h = relu(W1l@agg + W1r@self + b1); outputs z = (W2l@h)^T node-major bf16
    and r = W2r@h + b2 feature-major f32.
    """
    NCH = C_LO + C_HI
    nc = bacc.Bacc("TRN2", target_bir_lowering=False, debug=False,
                   enable_asserts=False, num_devices=N_CORES,
                   num_swdge_queues=4)
    dt = mybir.dt
    x_lo = nc.dram_tensor("x_lo", [SPLIT, P], dt.bfloat16, kind="ExternalInput").ap()
    x_hi = nc.dram_tensor("x_hi", [NPAD - SPLIT, P], dt.bfloat16, kind="ExternalInput").ap()
    selfT = nc.dram_tensor("selfT", [P, NPAD_CORE], dt.bfloat16, kind="ExternalInput").ap()
    w1lT = nc.dram_tensor("w1lT", [P, DIM_H], dt.bfloat16, kind="ExternalInput").ap()
    w1rT = nc.dram_tensor("w1rT", [P, DIM_H], dt.bfloat16, kind="ExternalInput").ap()
    w2lT = nc.dram_tensor("w2lT", [P, 2 * DIM_OUT], dt.bfloat16, kind="ExternalInput").ap()
    w2rT = nc.dram_tensor("w2rT", [P, 2 * DIM_OUT], dt.bfloat16, kind="ExternalInput").ap()
    b1c = nc.dram_tensor("b1c", [P, 2], dt.float32, kind="ExternalInput").ap()
    b2c = nc.dram_tensor("b2c", [P, 1], dt.float32, kind="ExternalInput").ap()
    seg = _seg_inputs(nc, dt, C_LO, C_HI)
    z_out = nc.dram_tensor("z_out", [NPAD_CORE, DIM_OUT], dt.bfloat16, kind="ExternalOutput").ap()
    r_out = nc.dram_tensor("r_out", [DIM_OUT, NPAD_CORE], dt.bfloat16, kind="ExternalOutput").ap()

    with tile.TileContext(nc) as tc:
        with ExitStack() as ctx:
            const = ctx.enter_context(tc.tile_pool(name="const", bufs=1))
            mlo = ctx.enter_context(tc.tile_pool(name="mlo", bufs=2))
            mhi = ctx.enter_context(tc.tile_pool(name="mhi", bufs=2))
            spool = ctx.enter_context(tc.tile_pool(name="spool", bufs=3))
            wk = ctx.enter_context(tc.tile_pool(name="wk", bufs=3))
            hp = ctx.enter_context(tc.tile_pool(name="hp", bufs=3))
            outp = ctx.enter_context(tc.tile_pool(name="outp", bufs=4))
            psA = ctx.enter_context(tc.tile_pool(name="psA", bufs=2, space="PSUM"))
            psB = ctx.enter_context(tc.tile_pool(name="psB", bufs=2, space="PSUM"))
            psC = ctx.enter_context(tc.tile_pool(name="psC", bufs=2, space="PSUM"))
            psD = ctx.enter_context(tc.tile_pool(name="psD", bufs=2, space="PSUM"))

            sb = _load_seg_consts(nc, const, dt, seg, C_LO, C_HI)
            selfT_sb = const.tile([P, NPAD_CORE], dt.bfloat16, name="selfT_sb")
            nc.sync.dma_start(selfT_sb[:], selfT[:, :])
            w1l_sb = const.tile([P, DIM_H], dt.bfloat16, name="w1l_sb")
            nc.sync.dma_start(w1l_sb[:], w1lT[:, :])
            w1r_sb = const.tile([P, DIM_H], dt.bfloat16, name="w1r_sb")
            nc.sync.dma_start(w1r_sb[:], w1rT[:, :])
            w2l_sb = const.tile([P, 2 * DIM_OUT], dt.bfloat16, name="w2l_sb")
            nc.sync.dma_start(w2l_sb[:], w2lT[:, :])
            w2r_sb = const.tile([P, 2 * DIM_OUT], dt.bfloat16, name="w2r_sb")
            nc.sync.dma_start(w2r_sb[:], w2rT[:, :])
            b1_sb = const.tile([P, 2], dt.float32, name="b1_sb")
            nc.sync.dma_start(b1_sb[:], b1c[:, :])
            b2_sb = const.tile([P, 1], dt.float32, name="b2_sb")
            nc.sync.dma_start(b2_sb[:], b2c[:, :])

            for g, gsz in enumerate(GROUPS):
                msgs_lo = mlo.tile([P, gsz * C_LO * P], dt.bfloat16)
                msgs_hi = mhi.tile([P, gsz * C_HI * P], dt.bfloat16)
                _gather_group(nc, msgs_lo, msgs_hi, x_lo, x_hi, sb, g, C_LO, C_HI)
                for j in range(gsz):
                    tt = GOFF[g] + j
                    S = spool.tile([P, NCH * P], dt.bfloat16)
                    _build_S(nc, S, sb, tt, C_LO, C_HI)
                    aggT_ps = psA.tile([P, P], dt.float32)
                    _seg_matmuls(nc, aggT_ps, msgs_lo, msgs_hi, S, j, C_LO, C_HI, P)
                    # mean: scale by 1/deg (per dst node = free dim), to bf16
                    aggT_sb = wk.tile([P, P], dt.bfloat16)
                    nc.vector.tensor_tensor(
                        out=aggT_sb[:], in0=aggT_ps[:],
                        in1=sb["recip"][:, tt * P:(tt + 1) * P],
                        op=mybir.AluOpType.mult)
                    # dense: hT[so] = relu(W1l@agg + W1r@self + b1), feature-major
                    hT = hp.tile([P, DIM_H], dt.bfloat16)
                    for so in range(2):
                        z1 = psB.tile([P, P], dt.float32)
                        nc.tensor.matmul(out=z1[:], lhsT=w1l_sb[:, so * P:(so + 1) * P],
                                         rhs=aggT_sb[:], start=True, stop=False)
                        nc.tensor.matmul(out=z1[:], lhsT=w1r_sb[:, so * P:(so + 1) * P],
                                         rhs=selfT_sb[:, tt * P:(tt + 1) * P],
                                         start=False, stop=True)
                        nc.scalar.activation(hT[:, so * P:(so + 1) * P], z1[:],
                                             mybir.ActivationFunctionType.Relu,
                                             bias=b1_sb[:, so:so + 1], scale=1.0)
                    # z = (W2l@h)^T node-major bf16 -> gather table rows
                    z_ps = psC.tile([P, DIM_OUT], dt.float32)
                    for so in range(2):
                        nc.tensor.matmul(out=z_ps[:], lhsT=hT[:, so * P:(so + 1) * P],
                                         rhs=w2l_sb[:, so * DIM_OUT:(so + 1) * DIM_OUT],
                                         start=(so == 0), stop=(so == 1))
                    z_sb = outp.tile([P, DIM_OUT], dt.bfloat16)
                    nc.scalar.copy(z_sb[:], z_ps[:])
                    nc.sync.dma_start(z_out[tt * P:(tt + 1) * P, :], z_sb[:])
                    # r = W2r@h + b2, feature-major f32
                    r_ps = psD.tile([DIM_OUT, P], dt.float32)
                    for so in range(2):
                        nc.tensor.matmul(out=r_ps[:], lhsT=w2r_sb[:, so * DIM_OUT:(so + 1) * DIM_OUT],
                                         rhs=hT[:, so * P:(so + 1) * P],
                                         start=(so == 0), stop=(so == 1))
                    r_sb = outp.tile([DIM_OUT, P], dt.bfloat16)
                    nc.vector.tensor_add(r_sb[:], r_ps[:],
                                         b2_sb[:DIM_OUT, 0:1].to_broadcast([DIM_OUT, P]))
                    nc.sync.dma_start(r_out[:, tt * P:(tt + 1) * P], r_sb[:])
    nc.compile()
    return nc


def _build_prog2(C_LO, C_HI):
    """Layer 2: gather z rows, segment-sum, mean-scale, add r."""
    NCH = C_LO + C_HI
    nc = bacc.Bacc("TRN2", target_bir_lowering=False, debug=False,
                   enable_asserts=False, num_devices=N_CORES,
                   num_swdge_queues=4)
    dt = mybir.dt
    z_lo = nc.dram_tensor("z_lo", [SPLIT, P], dt.bfloat16, kind="ExternalInput").ap()
    z_hi = nc.dram_tensor("z_hi", [NPAD - SPLIT, P], dt.bfloat16, kind="ExternalInput").ap()
    r_in = nc.dram_tensor("r_in", [DIM_OUT, NPAD_CORE], dt.bfloat16, kind="ExternalInput").ap()
    seg = _seg_inputs(nc, dt, C_LO, C_HI)
    outT = nc.dram_tensor("outT", [DIM_OUT, NPAD_CORE], dt.bfloat16, kind="ExternalOutput").ap()

    with tile.TileContext(nc) as tc:
        with ExitStack() as ctx:
            const = ctx.enter_context(tc.tile_pool(name="const", bufs=1))
            mlo = ctx.enter_context(tc.tile_pool(name="mlo", bufs=3))
            mhi = ctx.enter_context(tc.tile_pool(name="mhi", bufs=3))
            spool = ctx.enter_context(tc.tile_pool(name="spool", bufs=3))
            wk = ctx.enter_context(tc.tile_pool(name="wk", bufs=3))
            outp = ctx.enter_context(tc.tile_pool(name="outp", bufs=4))
            psA = ctx.enter_context(tc.tile_pool(name="psA", bufs=3, space="PSUM"))

            sb = _load_seg_consts(nc, const, dt, seg, C_LO, C_HI)
            r_sb = const.tile([DIM_OUT, NPAD_CORE], dt.bfloat16, name="r_sb")
            nc.sync.dma_start(r_sb[:], r_in[:, :])

            for g, gsz in enumerate(GROUPS):
                msgs_lo = mlo.tile([P, gsz * C_LO * P], dt.bfloat16)
                msgs_hi = mhi.tile([P, gsz * C_HI * P], dt.bfloat16)
                _gather_group(nc, msgs_lo, msgs_hi, z_lo, z_hi, sb, g, C_LO, C_HI)
                for j in range(gsz):
                    tt = GOFF[g] + j
                    S = spool.tile([P, NCH * P], dt.bfloat16)
                    _build_S(nc, S, sb, tt, C_LO, C_HI)
                    agg_ps = psA.tile([DIM_OUT, P], dt.float32)
                    _seg_matmuls(nc, agg_ps, msgs_lo, msgs_hi, S, j, C_LO, C_HI, DIM_OUT)
                    tmp = wk.tile([DIM_OUT, P], dt.float32)
                    nc.vector.tensor_tensor(
                        out=tmp[:], in0=agg_ps[:],
                        in1=sb["recip"][:DIM_OUT, tt * P:(tt + 1) * P],
                        op=mybir.AluOpType.mult)
                    o_sb = outp.tile([DIM_OUT, P], dt.bfloat16)
                    nc.vector.tensor_add(o_sb[:], tmp[:], r_sb[:, tt * P:(tt + 1) * P])
                    nc.sync.dma_start(outT[:, tt * P:(tt + 1) * P], o_sb[:])
    nc.compile()
    return nc


_PROG_CACHE = {}


def _get_programs(C_LO, C_HI):
    key = (C_LO, C_HI)
    if key not in _PROG_CACHE:
        _PROG_CACHE[key] = (_build_prog1(C_LO, C_HI), _build_prog2(C_LO, C_HI))
    return _PROG_CACHE[key]


def kernel(x, edge_index, W1l, W1r, b1, W2l, W2r, b2):
    global LAST_RESULTS
    LAST_RESULTS = []
    x = np.asarray(x, np.float32)
    src = np.asarray(edge_index[0], np.int64)
    dst = np.asarray(edge_index[1], np.int64)

    deg = np.bincount(dst, minlength=N_NODES)
    tile_of, slot_of = _partition_nodes(deg)
    pos_of = tile_of * P + slot_of
    src_pos = pos_of[src]
    C_LO, C_HI, idx_los, idx_his, dst_rels = _edge_layout(src_pos, dst, tile_of, slot_of)
    NCH = C_LO + C_HI
    l1, l2 = _get_programs(C_LO, C_HI)

    trace = bool(int(__import__("os").environ.get("BASS_TRACE", "0") or 0))
    tkw = dict(trace=True, tmpdir=None) if trace else {}

    # gather table: x rows at padded positions, bf16
    x_table = np.zeros((NPAD, P), BF16)
    x_table[pos_of] = x.astype(BF16)
    x_lo, x_hi = x_table[:SPLIT], x_table[SPLIT:]

    iota = np.ascontiguousarray(
        np.broadcast_to(np.tile(np.arange(P, dtype=np.float32), NCH), (P, NCH * P))
    ).astype(BF16)

    # per-core self features (feature-major) and 1/deg replicated tile
    selfTs, recips = [], []
    rec_full = np.ones(NPAD, np.float32)
    rec_full[pos_of] = 1.0 / np.maximum(deg, 1.0)
    sT_full = np.zeros((NPAD, P), np.float32)
    sT_full[pos_of] = x
    for c in range(N_CORES):
        sl = slice(c * NPAD_CORE, (c + 1) * NPAD_CORE)
        selfTs.append(np.ascontiguousarray(sT_full[sl].T.astype(BF16)))
        recips.append(np.ascontiguousarray(
            np.broadcast_to(rec_full[sl], (P, NPAD_CORE))).astype(np.float32))

    W1l, W1r, W2l, W2r = (np.asarray(a, np.float32) for a in (W1l, W1r, W2l, W2r))
    w1lT = np.ascontiguousarray(W1l.T).astype(BF16)
    w1rT = np.ascontiguousarray(W1r.T).astype(BF16)
    w2lT = np.ascontiguousarray(np.hstack([W2l.T[:P], W2l.T[P:]])).astype(BF16)
    w2rT = np.ascontiguousarray(np.hstack([W2r.T[:P], W2r.T[P:]])).astype(BF16)
    b1c = np.ascontiguousarray(np.asarray(b1, np.float32).reshape(2, P).T)
    b2c = np.zeros((P, 1), np.float32)
    b2c[:DIM_OUT, 0] = np.asarray(b2, np.float32)

    in_maps = []
    for c in range(N_CORES):
        in_maps.append({
            "x_lo": x_lo, "x_hi": x_hi,
            "selfT": selfTs[c],
            "w1lT": w1lT, "w1rT": w1rT, "w2lT": w2lT, "w2rT": w2rT,
            "b1c": b1c, "b2c": b2c,
            "idx_lo": idx_los[c], "idx_hi": idx_his[c],
            "dst_rel": dst_rels[c], "recip": recips[c], "iota": iota,
        })
    r1 = _run_spmd_retry(l1, in_maps, **tkw)
    LAST_RESULTS.append(r1)

    # assemble the replicated z gather table (bf16, padded to 128 cols)
    z_pad = np.zeros((NPAD, P), BF16)
    for c in range(N_CORES):
        z_pad[c * NPAD_CORE:(c + 1) * NPAD_CORE, :DIM_OUT] = \
            np.asarray(r1.results[c]["z_out"])
    z_lo, z_hi = z_pad[:SPLIT], z_pad[SPLIT:]

    in_maps2 = []
    for c in range(N_CORES):
        in_maps2.append({
            "z_lo": z_lo, "z_hi": z_hi,
            "r_in": np.asarray(r1.results[c]["r_out"]),
            "idx_lo": idx_los[c], "idx_hi": idx_his[c],
            "dst_rel": dst_rels[c], "recip": recips[c], "iota": iota,
        })
    r2 = _run_spmd_retry(l2, in_maps2, **tkw)
    LAST_RESULTS.append(r2)

    big = np.concatenate([np.asarray(r2.results[c]["outT"], np.float32)
                          for c in range(N_CORES)], axis=1)  # [64, NPAD]
    return np.ascontiguousarray(big[:, pos_of].T, dtype=np.float32)
